# revision 31
# baseline (speedup 1.0000x reference)
"""Trainium2 Bass kernel for nn_AttentionBlock (dense transformer block).

Strategy (8 NeuronCores, one chip):
  - Attention head-parallel: core c owns heads {2c, 2c+1} for all batches;
    computes Q/K/V projections (only its 2 heads), scores^T, softmax (exp on
    ScalarE, denominator via ones-column in the V matmul, DVE reciprocal),
    and the normalized context ctx^T.
  - AllToAll redistributes ctx^T shards so core c gets ALL heads for its
    S/8 = 256-query token shard.
  - Token-parallel back half: WO + residual + LN1 + MLP(relu) + residual +
    LN2 for the core's 1024 tokens (4 batches x 256 queries).
  All data flows feature-major ("transposed"): tiles are [feature_partition,
  token_free], so every matmul contraction runs on the partition dim.
  All matmuls use float32r (TF32-like, ~1.5e-4 relative error, ~4x the
  throughput of plain fp32 on TRN2's PE).
"""
import numpy as np

import concourse.bass as bass
import concourse.tile as tile
from concourse import mybir, bacc
from concourse.masks import make_identity

F32 = mybir.dt.float32
F32R = mybir.dt.float32r
BF16 = mybir.dt.bfloat16
AF = mybir.ActivationFunctionType
ALU = mybir.AluOpType

S, B, DM, H, DFF = 2048, 4, 1024, 16, 4096
DK = DM // H  # 64
NC = 8
SS = S // NC  # 256: seq shard per core
T = B * SS  # 1024 tokens per core in the back half
EPS = 1e-5

_CACHE = {}


def _build_nc():
    nc = bacc.Bacc("TRN2", target_bir_lowering=False, debug=False, num_devices=NC)

    # ---------------- I/O ----------------
    xt = nc.declare_dram_parameter("xt", [DM, B, S], BF16, isOutput=False)
    xts = nc.declare_dram_parameter("xts", [DM, B, SS], F32, isOutput=False)
    wqt = nc.declare_dram_parameter("wqt", [8, 128, 128], BF16, isOutput=False)
    wkt = nc.declare_dram_parameter("wkt", [8, 128, 128], BF16, isOutput=False)
    wvt = nc.declare_dram_parameter("wvt", [8, 128, 128], BF16, isOutput=False)
    bqkv = nc.declare_dram_parameter("bqkv", [128, 3], F32, isOutput=False)
    wotr = nc.declare_dram_parameter("wotr", [8, 8, 128, 128], BF16, isOutput=False)
    wob = nc.declare_dram_parameter("wob", [128, 8], F32, isOutput=False)
    w1tr = nc.declare_dram_parameter("w1tr", [32, 8, 128, 128], F32R, isOutput=False)
    b1 = nc.declare_dram_parameter("b1", [128, 32], F32, isOutput=False)
    w2tr = nc.declare_dram_parameter("w2tr", [8, 32, 128, 128], BF16, isOutput=False)
    b2 = nc.declare_dram_parameter("b2", [128, 8], F32, isOutput=False)
    ln1g = nc.declare_dram_parameter("ln1g", [128, 8], F32, isOutput=False)
    ln1b = nc.declare_dram_parameter("ln1b", [128, 8], F32, isOutput=False)
    ln2g = nc.declare_dram_parameter("ln2g", [128, 8], F32, isOutput=False)
    ln2b = nc.declare_dram_parameter("ln2b", [128, 8], F32, isOutput=False)
    ot = nc.declare_dram_parameter("ot", [DM, B, SS], F32, isOutput=True)

    with tile.TileContext(nc) as tc, nc.allow_low_precision(
        reason="float32r matmul operands (TF32-like) are intentional"
    ):
        with (
            tc.tile_pool(name="dram", bufs=1, space="DRAM") as dram,
            tc.tile_pool(name="const", bufs=1) as const,
            tc.tile_pool(name="wres", bufs=1) as wres,
            tc.tile_pool(name="qkv", bufs=1) as qkvp,
            tc.tile_pool(name="gen", bufs=2) as gen,
            tc.tile_pool(name="ps", bufs=2, space="PSUM") as ps,
        ):
            # warm start: first attention rhs tiles, issued before anything
            warm_xt = []
            for ic in range(8):
                x_ = gen.tile([128, 512], BF16, tag="xtin", bufs=8,
                              name=f"warm_xt{ic}")
                nc.sync.dma_start(
                    out=x_[:], in_=xt[ic * 128:(ic + 1) * 128, 0, 0:512])
                warm_xt.append(x_)

            a2a_in = []
            a2a_out = []
            for b in range(B):
                ai = dram.tile([NC, 2, DK, SS], BF16, tag=f"a2a_in{b}",
                               name=f"a2a_in{b}")
                ao = dram.tile([NC, 2, DK, SS], BF16, tag=f"a2a_out{b}",
                               name=f"a2a_out{b}")
                a2a_in.append(ai)
                a2a_out.append(ao)

            # ---------------- constants ----------------
            ident = const.tile([128, 128], BF16, tag="ident")
            make_identity(nc, ident[:])
            ones_1x64 = const.tile([1, 64], BF16, tag="ones_1x64")
            nc.gpsimd.memset(ones_1x64[:], 1.0)
            ones_f32 = const.tile([128, 128], F32, tag="ones_f32")
            nc.vector.memset(ones_f32[:], 1.0)
            ones_1x128 = const.tile([1, 128], F32R, tag="ones_1x128")
            nc.vector.tensor_copy(out=ones_1x128[:], in_=ones_f32[0:1, :])
            ones_128x1 = const.tile([128, 1], F32R, tag="ones_128x1")
            nc.vector.tensor_copy(out=ones_128x1[:], in_=ones_f32[:, 0:1])
            eps_sb = const.tile([1, 1], F32, tag="eps")
            nc.vector.memset(eps_sb[:], EPS)
            bqkv_sb = const.tile([128, 3], F32, tag="bqkv")
            nc.sync.dma_start(out=bqkv_sb[:], in_=bqkv[:, :])
            wob_sb = const.tile([128, 8], F32, tag="wob")
            nc.sync.dma_start(out=wob_sb[:], in_=wob[:, :])
            b1_sb = const.tile([128, 32], F32, tag="b1")
            nc.sync.dma_start(out=b1_sb[:], in_=b1[:, :])
            b2_sb = const.tile([128, 8], F32, tag="b2")
            nc.sync.dma_start(out=b2_sb[:], in_=b2[:, :])
            ln_sb = {}
            for name, h in (("ln1g", ln1g), ("ln1b", ln1b), ("ln2g", ln2g),
                            ("ln2b", ln2b)):
                t_ = const.tile([128, 8], F32, tag=name)
                nc.sync.dma_start(out=t_[:], in_=h[:, :])
                ln_sb[name] = t_

            # resident QKV weights: [128p, 8ic, 128(2h dk)]
            w_sb = {}
            for name, h in (("wq", wqt), ("wk", wkt), ("wv", wvt)):
                t_ = wres.tile([128, 8, 128], BF16, tag=name)
                nc.sync.dma_start(
                    out=t_[:], in_=h[:, :, :].rearrange("ic p j -> p ic j")
                )
                w_sb[name] = t_

            # early prefetch of back-half weight streams (no data deps):
            # first tiles of wo/w1/w2 so MLP never cold-starts on DMA.
            pre_wo = {}
            pre_w1 = {}
            pre_w2 = {}
            for oc in range(2):
                t_ = gen.tile([128, 8, 128], BF16, tag="wo_c", bufs=2,
                              name=f"pre_wo{oc}")
                nc.gpsimd.dma_start(
                    out=t_[:],
                    in_=wotr[oc, :, :, :].rearrange("ic p j -> p ic j"))
                pre_wo[oc] = t_
            for fc in range(3):
                t_ = gen.tile([128, 8, 128], F32R, tag="w1_c", bufs=3,
                              name=f"pre_w1{fc}")
                nc.gpsimd.dma_start(
                    out=t_[:],
                    in_=w1tr[fc, :, :, :].rearrange("ic p j -> p ic j"))
                pre_w1[fc] = t_
            for half in range(2):
                t_ = gen.tile([128, 16, 128], BF16, tag="w2_c", bufs=2,
                              name=f"pre_w2{half}")
                nc.gpsimd.dma_start(
                    out=t_[:],
                    in_=w2tr[0, half * 16:(half + 1) * 16, :, :]
                    .rearrange("fc p j -> p fc j"))
                pre_w2[half] = t_

            # persistent V^T tiles: 16 l-chunks of [128l, 2*65]
            vt_tiles = []
            for lc in range(16):
                v_ = const.tile([128, 130], BF16, tag=f"vt{lc}")
                nc.gpsimd.memset(v_[:, 64:65], 1.0)
                nc.gpsimd.memset(v_[:, 129:130], 1.0)
                vt_tiles.append(v_)

            # =====================================================
            # Attention phase, per batch b
            # =====================================================
            for b in range(B):
                # ---- P1: Q/K/V projection for this core's 2 heads ----
                proj = {}
                for name in ("wq", "wk", "wv"):
                    proj[name] = qkvp.tile([128, S], BF16, tag=f"{name}_out",
                                           name=f"{name}_out_b{b}")
                for sc in range(4):
                    if b == 0 and sc == 0:
                        xt_t = warm_xt
                    else:
                        xt_t = []
                        for ic in range(8):
                            x_ = gen.tile([128, 512], BF16, tag="xtin", bufs=8)
                            nc.sync.dma_start(
                                out=x_[:],
                                in_=xt[ic * 128:(ic + 1) * 128, b,
                                       sc * 512:(sc + 1) * 512],
                            )
                            xt_t.append(x_)
                    for pi, name in enumerate(("wq", "wk", "wv")):
                        pmm = ps.tile([128, 512], F32, tag="pb2", bufs=3,
                                      padded_shape=[128, 1024])
                        for ic in range(8):
                            nc.tensor.matmul(
                                out=pmm[:],
                                lhsT=w_sb[name][:, ic, :],
                                rhs=xt_t[ic][:],
                                start=(ic == 0),
                                stop=(ic == 7),
                            )
                        # evict + bias (+implicit bf16 round)
                        nc.vector.tensor_scalar_add(
                            out=proj[name][:, sc * 512:(sc + 1) * 512],
                            in0=pmm[:],
                            scalar1=bqkv_sb[:, pi:pi + 1],
                        )

                # ---- P2: transpose V -> vt_tiles [l, (hl,v)] ----
                for lc in range(16):
                    ptr = ps.tile([128, 128], BF16, tag="pb1", bufs=2,
                                  padded_shape=[128, 512])
                    nc.tensor.transpose(
                        out=ptr[:],
                        in_=proj["wv"][:, lc * 128:(lc + 1) * 128],
                        identity=ident[:],
                    )
                    # scatter (hl,v) -> free offset hl*65+v
                    for hl in range(2):
                        nc.vector.tensor_copy(
                            out=vt_tiles[lc][:, hl * 65:hl * 65 + 64],
                            in_=ptr[:, hl * 64:(hl + 1) * 64],
                        )

                # ---- P3: attention per head ----
                for hl in range(2):
                    hb = hl * 64
                    for sc in range(4):
                        exp_t = []
                        for lc2 in range(8):
                            psc = ps.tile([128, 1024], F32, tag="pb2", bufs=3)
                            for k in range(2):
                                nc.tensor.matmul(
                                    out=psc[:, k * 512:(k + 1) * 512],
                                    lhsT=proj["wk"][hb:hb + 64,
                                                    (lc2 * 2 + k) * 128:
                                                    (lc2 * 2 + k + 1) * 128],
                                    rhs=proj["wq"][hb:hb + 64,
                                                   sc * 512:(sc + 1) * 512],
                                    start=True,
                                    stop=True,
                                )
                            e_ = gen.tile([128, 1024], BF16, tag="u2k",
                                          bufs=24)
                            nc.scalar.activation(
                                out=e_[:], in_=psc[:], func=AF.Exp
                            )
                            exp_t.append(e_)
                        pctx = ps.tile([65, 512], F32, tag="pb1", bufs=2,
                                       padded_shape=[128, 512])
                        for lc in range(16):
                            nc.tensor.matmul(
                                out=pctx[:],
                                lhsT=vt_tiles[lc][:, hl * 65:hl * 65 + 65],
                                rhs=exp_t[lc // 2][:, (lc % 2) * 512:
                                                   (lc % 2 + 1) * 512],
                                start=(lc == 0),
                                stop=(lc == 15),
                            )
                        dsum = gen.tile([1, 512], F32, tag="dsum", bufs=1)
                        nc.vector.tensor_copy(out=dsum[:], in_=pctx[64:65, :])
                        rf32 = gen.tile([1, 512], F32, tag="rf32", bufs=1)
                        nc.vector.reciprocal_approx_fast(
                            out=rf32[:], in_=dsum[:])
                        recip = gen.tile([1, 512], BF16, tag="recip", bufs=2)
                        nc.vector.tensor_copy(out=recip[:], in_=rf32[:])
                        pbc = ps.tile([64, 512], F32, tag="pb1", bufs=2,
                                      padded_shape=[128, 512])
                        nc.tensor.matmul(
                            out=pbc[:], lhsT=ones_1x64[:], rhs=recip[:],
                            start=True, stop=True,
                        )
                        bc_sb = gen.tile([64, 512], F32, tag="bc_sb", bufs=2)
                        nc.vector.tensor_copy(out=bc_sb[:], in_=pbc[:])
                        stage = gen.tile([64, 512], BF16, tag="stage", bufs=2)
                        nc.vector.tensor_tensor(
                            out=stage[:], in0=pctx[0:64, :], in1=bc_sb[:],
                            op=ALU.mult,
                        )
                        # write the two destination shards
                        for half in range(2):
                            d = sc * 2 + half
                            nc.sync.dma_start(
                                out=a2a_in[b][d, hl, :, :],
                                in_=stage[:, half * 256:(half + 1) * 256],
                            )

                # A2A for this batch (overlaps later batches' attention)
                nc.gpsimd.collective_compute(
                    "AllToAll",
                    ALU.bypass,
                    replica_groups=[list(range(NC))],
                    ins=[a2a_in[b][:].opt()],
                    outs=[a2a_out[b][:].opt()],
                )



            # =====================================================
            # P5: back half on the core's 1024 tokens (2 chunks of 512)
            # =====================================================
            def layernorm(rt_tiles, g_sb, b_sb, out_dtype, out_tag,
                          fold_gb=False):
                """rt_tiles: 8 tiles [128,512] (f32r) of pre-LN activations.
                Returns 8 tiles [128,512] of LN output (dtype out_dtype)."""
                psum_s = ps.tile([1, 512], F32, tag="pb1", bufs=2,
                                 padded_shape=[128, 512])
                for ic in range(8):
                    nc.tensor.matmul(
                        out=psum_s[:], lhsT=ones_128x1[:], rhs=rt_tiles[ic][:],
                        start=(ic == 0), stop=(ic == 7),
                    )
                sq_t = []
                for ic in range(8):
                    sq = gen.tile([128, 512], F32R, tag="sq", bufs=2)
                    nc.scalar.activation(out=sq[:], in_=rt_tiles[ic][:],
                                         func=AF.Square)
                    sq_t.append(sq)
                psum_q = ps.tile([1, 512], F32, tag="pb1", bufs=2,
                                 padded_shape=[128, 512])
                for ic in range(8):
                    nc.tensor.matmul(
                        out=psum_q[:], lhsT=ones_128x1[:], rhs=sq_t[ic][:],
                        start=(ic == 0), stop=(ic == 7),
                    )
                mu = gen.tile([1, 512], F32R, tag="mu", bufs=2)
                nc.vector.tensor_scalar_mul(out=mu[:], in0=psum_s[:],
                                            scalar1=1.0 / DM)
                ex2 = gen.tile([1, 512], F32, tag="stat", bufs=3)
                nc.vector.tensor_scalar_mul(out=ex2[:], in0=psum_q[:],
                                            scalar1=1.0 / DM)
                # var = ex2 - mu^2
                musq = gen.tile([1, 512], F32, tag="stat", bufs=3)
                nc.vector.tensor_tensor(out=musq[:], in0=mu[:], in1=mu[:],
                                        op=ALU.mult)
                var = gen.tile([1, 512], F32, tag="stat", bufs=3)
                nc.vector.tensor_tensor(out=var[:], in0=ex2[:], in1=musq[:],
                                        op=ALU.subtract)
                sd = gen.tile([1, 512], F32, tag="stat", bufs=3)
                nc.scalar.activation(out=sd[:], in_=var[:], func=AF.Sqrt,
                                     bias=eps_sb[:])
                rscr = gen.tile([1, 512], F32, tag="stat", bufs=3)
                rf = gen.tile([1, 512], F32, tag="stat", bufs=3)
                nc.vector.reciprocal_approx_accurate(out=rf[:], in_=sd[:],
                                                     scratch=rscr[:])
                rstd = gen.tile([1, 512], F32R, tag="rstd", bufs=2)
                nc.vector.tensor_copy(out=rstd[:], in_=rf[:])
                # broadcast mu, rstd across 128 partitions
                pmu = ps.tile([128, 512], F32, tag="pb1", bufs=2)
                nc.tensor.matmul(out=pmu[:], lhsT=ones_1x128[:], rhs=mu[:],
                                 start=True, stop=True)
                prs = ps.tile([128, 512], F32, tag="pb1", bufs=2)
                nc.tensor.matmul(out=prs[:], lhsT=ones_1x128[:], rhs=rstd[:],
                                 start=True, stop=True)
                outs = []
                for ic in range(8):
                    tmp = gen.tile([128, 512], F32, tag="lnw", bufs=4)
                    nc.vector.tensor_tensor(out=tmp[:], in0=rt_tiles[ic][:],
                                            in1=pmu[:], op=ALU.subtract)
                    if fold_gb:
                        # g/b are folded into the consumer weights; just
                        # z = (r - mu) * rstd
                        o_ = gen.tile([128, 512], out_dtype, tag="u2k",
                                      bufs=24, name=f"z_{ic}")
                        nc.vector.tensor_tensor(out=o_[:], in0=tmp[:],
                                                in1=prs[:], op=ALU.mult)
                    else:
                        a_ = gen.tile([128, 512], F32, tag="lnw", bufs=4)
                        nc.scalar.activation(out=a_[:], in_=prs[:],
                                             func=AF.Copy,
                                             scale=g_sb[:, ic:ic + 1])
                        t2 = gen.tile([128, 512], F32, tag="lnw", bufs=4)
                        nc.vector.tensor_tensor(out=t2[:], in0=tmp[:],
                                                in1=a_[:], op=ALU.mult)
                        o_ = gen.tile([128, 512], out_dtype, tag="u2k",
                                      bufs=24)
                        nc.scalar.activation(out=o_[:], in_=t2[:],
                                             func=AF.Identity,
                                             bias=b_sb[:, ic:ic + 1])
                    outs.append(o_)
                return outs

            for t2c in range(2):  # token chunk: 512 tokens (2 batches)
                b0 = t2c * 2
                # ctx tiles [128i, 512t]
                ctx_t = []
                for ic in range(8):
                    c_ = gen.tile([128, 512], BF16, tag="ctx", bufs=8)
                    for half in range(2):
                        nc.sync.dma_start(
                            out=c_[:, half * 256:(half + 1) * 256],
                            in_=a2a_out[b0 + half][ic, :, :, :].rearrange(
                                "hl v s -> (hl v) s"
                            ),
                        )
                    ctx_t.append(c_)
                # WO + residual -> R1
                r1_t = []
                for oc in range(8):
                    if t2c == 0 and oc in pre_wo:
                        wo_c = pre_wo.pop(oc)
                    else:
                        wo_c = gen.tile([128, 8, 128], BF16, tag="wo_c",
                                        bufs=2, name=f"wo_c_{t2c}_{oc}")
                        nc.sync.dma_start(
                            out=wo_c[:],
                            in_=wotr[oc, :, :, :].rearrange("ic p j -> p ic j"),
                        )
                    pmm = ps.tile([128, 512], F32, tag="pb2", bufs=3,
                                  padded_shape=[128, 1024])
                    for ic in range(8):
                        nc.tensor.matmul(
                            out=pmm[:], lhsT=wo_c[:, ic, :], rhs=ctx_t[ic][:],
                            start=(ic == 0), stop=(ic == 7),
                        )
                    x_ = gen.tile([128, 512], F32, tag="xres", bufs=3)
                    nc.sync.dma_start(
                        out=x_[:],
                        in_=xts[oc * 128:(oc + 1) * 128, b0:b0 + 2, :].rearrange(
                            "p b s -> p (b s)"
                        ),
                    )
                    r1 = gen.tile([128, 512], F32R, tag="u2k", bufs=24)
                    nc.vector.scalar_tensor_tensor(
                        out=r1[:], in0=pmm[:], scalar=wob_sb[:, oc:oc + 1],
                        in1=x_[:], op0=ALU.add, op1=ALU.add,
                    )
                    r1_t.append(r1)
                # LN1
                x1_t = layernorm(r1_t, None, None, F32R, "x1", fold_gb=True)
                # MLP up + relu
                h_t = []
                for fc in range(32):
                    if t2c == 0 and fc in pre_w1:
                        w1_c = pre_w1.pop(fc)
                    else:
                        w1_c = gen.tile([128, 8, 128], F32R, tag="w1_c",
                                        bufs=3, name=f"w1_c_{t2c}_{fc}")
                        nc.sync.dma_start(
                            out=w1_c[:],
                            in_=w1tr[fc, :, :, :].rearrange("ic p j -> p ic j"),
                        )
                    pmm = ps.tile([128, 512], F32, tag="pb2", bufs=3,
                                  padded_shape=[128, 1024])
                    for ic in range(8):
                        nc.tensor.matmul(
                            out=pmm[:], lhsT=w1_c[:, ic, :], rhs=x1_t[ic][:],
                            start=(ic == 0), stop=(ic == 7),
                        )
                    h_ = gen.tile([128, 512], BF16, tag="ht", bufs=33)
                    nc.scalar.activation(out=h_[:], in_=pmm[:], func=AF.Relu,
                                         bias=b1_sb[:, fc:fc + 1])
                    h_t.append(h_)
                # MLP down + residual -> R2
                r2_t = []
                for oc in range(8):
                    pmm = ps.tile([128, 512], F32, tag="pb2", bufs=3,
                                  padded_shape=[128, 1024])
                    for half in range(2):
                        if t2c == 0 and oc == 0 and half in pre_w2:
                            w2_c = pre_w2.pop(half)
                        else:
                            w2_c = gen.tile([128, 16, 128], BF16, tag="w2_c",
                                            bufs=2, name=f"w2_c_{t2c}_{oc}_{half}")
                            nc.sync.dma_start(
                                out=w2_c[:],
                                in_=w2tr[oc, half * 16:(half + 1) * 16, :, :]
                                .rearrange("fc p j -> p fc j"),
                            )
                        for f2 in range(16):
                            fc = half * 16 + f2
                            nc.tensor.matmul(
                                out=pmm[:], lhsT=w2_c[:, f2, :],
                                rhs=h_t[fc][:],
                                start=(fc == 0), stop=(fc == 31),
                            )
                    r2p = gen.tile([128, 512], F32, tag="lnw", bufs=4)
                    nc.vector.scalar_tensor_tensor(
                        out=r2p[:], in0=x1_t[oc][:],
                        scalar=ln_sb["ln1g"][:, oc:oc + 1],
                        in1=pmm[:], op0=ALU.mult, op1=ALU.add,
                    )
                    r2 = gen.tile([128, 512], F32R, tag="u2k", bufs=24)
                    nc.vector.tensor_scalar_add(
                        out=r2[:], in0=r2p[:], scalar1=b2_sb[:, oc:oc + 1],
                    )
                    r2_t.append(r2)
                # LN2 -> output
                o_t = layernorm(r2_t, ln_sb["ln2g"], ln_sb["ln2b"], F32, "otile")
                for oc in range(8):
                    nc.sync.dma_start(
                        out=ot[oc * 128:(oc + 1) * 128, b0:b0 + 2, :].rearrange(
                            "p b s -> p (b s)"
                        ),
                        in_=o_t[oc][:],
                    )

    nc.compile()
    return nc


# ------------------------------------------------------------------
# Host side
# ------------------------------------------------------------------
def _get_runner():
    if "runner" in _CACHE:
        return _CACHE["runner"]
    import jax
    from jax.sharding import Mesh, PartitionSpec
    try:
        from jax.experimental.shard_map import shard_map
    except ImportError:
        from jax.shard_map import shard_map
    from concourse import bass2jax
    from concourse.bass2jax import _bass_exec_p, install_neuronx_cc_hook

    nc = _build_nc()
    install_neuronx_cc_hook()
    partition_name = nc.partition_id_tensor.name if nc.partition_id_tensor else None
    in_names, out_names, out_avals, zero_outs = [], [], [], []
    for alloc in nc.m.functions[0].allocations:
        if not isinstance(alloc, mybir.MemoryLocationSet):
            continue
        name = alloc.memorylocations[0].name
        if alloc.kind == "ExternalInput":
            if name != partition_name:
                in_names.append(name)
        elif alloc.kind == "ExternalOutput":
            out_names.append(name)
            shape = tuple(alloc.tensor_shape)
            dtype = mybir.dt.np(alloc.dtype)
            out_avals.append(jax.core.ShapedArray(shape, dtype))
            zero_outs.append(np.zeros(shape, dtype))
    n_params = len(in_names)
    all_in_names = list(in_names) + list(out_names)
    if partition_name is not None:
        all_in_names.append(partition_name)

    def _body(*args):
        operands = list(args)
        if partition_name is not None:
            operands.append(bass2jax.partition_id_tensor())
        outs = _bass_exec_p.bind(
            *operands,
            out_avals=tuple(out_avals),
            in_names=tuple(all_in_names),
            out_names=tuple(out_names),
            lowering_input_output_aliases=(),
            sim_require_finite=True,
            sim_require_nnan=True,
            nc=nc,
        )
        return tuple(outs)

    donate = tuple(range(n_params, n_params + len(out_names)))
    devices = jax.devices()[:NC]
    mesh = Mesh(np.asarray(devices), ("core",))
    in_specs = (PartitionSpec("core"),) * (n_params + len(out_names))
    out_specs = (PartitionSpec("core"),) * len(out_names)
    fn = jax.jit(
        shard_map(_body, mesh=mesh, in_specs=in_specs, out_specs=out_specs,
                  check_rep=False),
        donate_argnums=donate, keep_unused=True,
    )

    class R:
        pass

    r = R()
    r.fn = fn
    r.in_names = in_names
    r.out_names = out_names
    r.out_avals = out_avals
    _CACHE["runner"] = r
    return r


def _prep_in_maps(X, WQ_w, WQ_b, WK_w, WK_b, WV_w, WV_b, WO_w, WO_b,
                  ln1_g, ln1_b, W1, b1, W2, b2, ln2_g, ln2_b):
    import ml_dtypes
    f = np.float32
    bf = ml_dtypes.bfloat16
    XT = np.ascontiguousarray(X.transpose(2, 1, 0)).astype(f)  # [DM,B,S]
    wotr = np.ascontiguousarray(
        WO_w.reshape(8, 128, 8, 128).transpose(0, 2, 3, 1)).astype(bf)
    W1f = (W1 * ln1_g[None, :]).astype(np.float64)
    b1f = (b1 + W1 @ ln1_b).astype(f)
    w1tr = np.ascontiguousarray(
        W1f.reshape(32, 128, 8, 128).transpose(0, 2, 3, 1)).astype(f)
    w2tr = np.ascontiguousarray(
        W2.reshape(8, 128, 32, 128).transpose(0, 2, 3, 1)).astype(bf)
    wob_t = np.ascontiguousarray(WO_b.reshape(8, 128).T).astype(f)
    b1_t = np.ascontiguousarray(b1f.reshape(32, 128).T).astype(f)
    b2f = (b2 + ln1_b).astype(f)
    b2_t = np.ascontiguousarray(b2f.reshape(8, 128).T).astype(f)
    ln1g_t = np.ascontiguousarray(ln1_g.reshape(8, 128).T).astype(f)
    ln1b_t = np.ascontiguousarray(ln1_b.reshape(8, 128).T).astype(f)
    ln2g_t = np.ascontiguousarray(ln2_g.reshape(8, 128).T).astype(f)
    ln2b_t = np.ascontiguousarray(ln2_b.reshape(8, 128).T).astype(f)

    in_maps = []
    for c in range(NC):
        h0 = 2 * c
        # [2,DK,DM] -> [DM, 128]: W2h[j, hl*64+k] = W[h0+hl, k, j]
        wq2 = WQ_w[h0:h0 + 2].reshape(128, DM).T / 8.0
        wk2 = WK_w[h0:h0 + 2].reshape(128, DM).T
        wv2 = WV_w[h0:h0 + 2].reshape(128, DM).T
        # [8,128,128] layout: [ic, p, j] = W2h[ic*128+p, j]
        wqt = np.ascontiguousarray(wq2.reshape(8, 128, 128)).astype(bf)
        wkt = np.ascontiguousarray(wk2.reshape(8, 128, 128)).astype(bf)
        wvt = np.ascontiguousarray(wv2.reshape(8, 128, 128)).astype(bf)
        bq = WQ_b[h0:h0 + 2].reshape(128) / 8.0
        bk = WK_b[h0:h0 + 2].reshape(128)
        bv = WV_b[h0:h0 + 2].reshape(128)
        bqkv = np.stack([bq, bk, bv], axis=1).astype(f)
        in_maps.append({
            "xt": XT.astype(bf),
            "xts": np.ascontiguousarray(XT[:, :, c * SS:(c + 1) * SS]),
            "wqt": wqt, "wkt": wkt, "wvt": wvt, "bqkv": bqkv,
            "wotr": wotr, "wob": wob_t,
            "w1tr": w1tr, "b1": b1_t, "w2tr": w2tr, "b2": b2_t,
            "ln1g": ln1g_t, "ln1b": ln1b_t, "ln2g": ln2g_t, "ln2b": ln2b_t,
        })
    return in_maps


def run_in_maps(in_maps):
    """Run the compiled kernel on prepared in_maps; returns list of out dicts."""
    import jax
    r = _get_runner()
    n = NC
    per_core = [[np.asarray(m[name]) for name in r.in_names] for m in in_maps]
    concat_in = [
        np.concatenate([per_core[c][i] for c in range(n)], axis=0)
        for i in range(len(r.in_names))
    ]
    concat_zeros = [
        np.zeros((n * a.shape[0], *a.shape[1:]), a.dtype) for a in r.out_avals
    ]
    out_arrs = r.fn(*concat_in, *concat_zeros)
    out_arrs = [np.asarray(a) for a in out_arrs]
    return [
        {name: out_arrs[i].reshape(n, *r.out_avals[i].shape)[c]
         for i, name in enumerate(r.out_names)}
        for c in range(n)
    ]


def kernel(**inputs):
    in_maps = _prep_in_maps(**inputs)
    results = run_in_maps(in_maps)
    # assemble: each core's ot is [DM, B, SS] covering s in [c*SS,(c+1)*SS)
    ot_full = np.concatenate([results[c]["ot"] for c in range(NC)], axis=2)
    # [DM, B, S] -> [S, B, DM]
    return np.ascontiguousarray(ot_full.transpose(2, 1, 0))


# revision 35
# speedup vs baseline: 1.0445x; 1.0445x over previous
"""Trainium2 Bass kernel for nn_AttentionBlock (dense transformer block).

Strategy (8 NeuronCores, one chip):
  - Attention head-parallel: core c owns heads {2c, 2c+1} for all batches;
    computes Q/K/V projections (only its 2 heads), scores^T, softmax (exp on
    ScalarE, denominator via ones-column in the V matmul, DVE reciprocal),
    and the normalized context ctx^T.
  - AllToAll redistributes ctx^T shards so core c gets ALL heads for its
    S/8 = 256-query token shard.
  - Token-parallel back half: WO + residual + LN1 + MLP(relu) + residual +
    LN2 for the core's 1024 tokens (4 batches x 256 queries).
  All data flows feature-major ("transposed"): tiles are [feature_partition,
  token_free], so every matmul contraction runs on the partition dim.
  All matmuls use float32r (TF32-like, ~1.5e-4 relative error, ~4x the
  throughput of plain fp32 on TRN2's PE).
"""
import numpy as np

import concourse.bass as bass
import concourse.tile as tile
from concourse import mybir, bacc
from concourse.masks import make_identity

F32 = mybir.dt.float32
F32R = mybir.dt.float32r
BF16 = mybir.dt.bfloat16
AF = mybir.ActivationFunctionType
ALU = mybir.AluOpType

S, B, DM, H, DFF = 2048, 4, 1024, 16, 4096
DK = DM // H  # 64
NC = 8
SS = S // NC  # 256: seq shard per core
T = B * SS  # 1024 tokens per core in the back half
EPS = 1e-5

_CACHE = {}


def _build_nc():
    nc = bacc.Bacc("TRN2", target_bir_lowering=False, debug=False, num_devices=NC)

    # ---------------- I/O ----------------
    xt = nc.declare_dram_parameter("xt", [DM, B, S], BF16, isOutput=False)
    xts = nc.declare_dram_parameter("xts", [DM, B, SS], F32, isOutput=False)
    wqt = nc.declare_dram_parameter("wqt", [8, 128, 128], BF16, isOutput=False)
    wkt = nc.declare_dram_parameter("wkt", [8, 128, 128], BF16, isOutput=False)
    wvt = nc.declare_dram_parameter("wvt", [8, 128, 128], BF16, isOutput=False)
    bqkv = nc.declare_dram_parameter("bqkv", [128, 3], F32, isOutput=False)
    wotr = nc.declare_dram_parameter("wotr", [8, 8, 128, 128], BF16, isOutput=False)
    wob = nc.declare_dram_parameter("wob", [128, 8], F32, isOutput=False)
    w1tr = nc.declare_dram_parameter("w1tr", [32, 8, 128, 128], F32R, isOutput=False)
    b1 = nc.declare_dram_parameter("b1", [128, 32], F32, isOutput=False)
    w2tr = nc.declare_dram_parameter("w2tr", [8, 32, 128, 128], BF16, isOutput=False)
    b2 = nc.declare_dram_parameter("b2", [128, 8], F32, isOutput=False)
    ln1g = nc.declare_dram_parameter("ln1g", [128, 8], F32, isOutput=False)
    ln1b = nc.declare_dram_parameter("ln1b", [128, 8], F32, isOutput=False)
    ln2g = nc.declare_dram_parameter("ln2g", [128, 8], F32, isOutput=False)
    ln2b = nc.declare_dram_parameter("ln2b", [128, 8], F32, isOutput=False)
    ot = nc.declare_dram_parameter("ot", [DM, B, SS], F32, isOutput=True)

    with tile.TileContext(nc) as tc, nc.allow_low_precision(
        reason="float32r matmul operands (TF32-like) are intentional"
    ):
        with (
            tc.tile_pool(name="dram", bufs=1, space="DRAM") as dram,
            tc.tile_pool(name="const", bufs=1) as const,
            tc.tile_pool(name="wres", bufs=1) as wres,
            tc.tile_pool(name="qkv", bufs=1) as qkvp,
            tc.tile_pool(name="gen", bufs=2) as gen,
            tc.tile_pool(name="ps", bufs=2, space="PSUM") as ps,
        ):
            a2a_in = []
            a2a_out = []
            for b in range(B):
                ai = dram.tile([NC, 2, DK, SS], BF16, tag=f"a2a_in{b}",
                               name=f"a2a_in{b}")
                ao = dram.tile([NC, 2, DK, SS], BF16, tag=f"a2a_out{b}",
                               name=f"a2a_out{b}")
                a2a_in.append(ai)
                a2a_out.append(ao)

            # ---------------- constants ----------------
            ident = const.tile([128, 128], BF16, tag="ident")
            make_identity(nc, ident[:])
            ones_1x64 = const.tile([1, 64], BF16, tag="ones_1x64")
            nc.gpsimd.memset(ones_1x64[:], 1.0)
            ones_f32 = const.tile([128, 128], F32, tag="ones_f32")
            nc.vector.memset(ones_f32[:], 1.0)
            ones_1x128 = const.tile([1, 128], F32R, tag="ones_1x128")
            nc.vector.tensor_copy(out=ones_1x128[:], in_=ones_f32[0:1, :])
            ones_128x1 = const.tile([128, 1], F32R, tag="ones_128x1")
            nc.vector.tensor_copy(out=ones_128x1[:], in_=ones_f32[:, 0:1])
            eps_sb = const.tile([1, 1], F32, tag="eps")
            nc.vector.memset(eps_sb[:], EPS)
            bqkv_sb = const.tile([128, 3], F32, tag="bqkv")
            nc.sync.dma_start(out=bqkv_sb[:], in_=bqkv[:, :])
            wob_sb = const.tile([128, 8], F32, tag="wob")
            nc.sync.dma_start(out=wob_sb[:], in_=wob[:, :])
            b1_sb = const.tile([128, 32], F32, tag="b1")
            nc.sync.dma_start(out=b1_sb[:], in_=b1[:, :])
            b2_sb = const.tile([128, 8], F32, tag="b2")
            nc.sync.dma_start(out=b2_sb[:], in_=b2[:, :])
            ln_sb = {}
            for name, h in (("ln1g", ln1g), ("ln1b", ln1b), ("ln2g", ln2g),
                            ("ln2b", ln2b)):
                t_ = const.tile([128, 8], F32, tag=name)
                nc.sync.dma_start(out=t_[:], in_=h[:, :])
                ln_sb[name] = t_

            # resident QKV weights: [128p, 8ic, 128(2h dk)]
            w_sb = {}
            for name, h in (("wq", wqt), ("wk", wkt), ("wv", wvt)):
                t_ = wres.tile([128, 8, 128], BF16, tag=name)
                nc.sync.dma_start(
                    out=t_[:], in_=h[:, :, :].rearrange("ic p j -> p ic j")
                )
                w_sb[name] = t_
            # warm start: first attention rhs tiles, issued before anything
            warm_xt = []
            for ic in range(8):
                x_ = gen.tile([128, 512], BF16, tag="xtin", bufs=8,
                              name=f"warm_xt{ic}")
                nc.sync.dma_start(
                    out=x_[:], in_=xt[ic * 128:(ic + 1) * 128, 0, 0:512])
                warm_xt.append(x_)


            # early prefetch of back-half weight streams (no data deps):
            # first tiles of wo/w1/w2 so MLP never cold-starts on DMA.
            pre_wo = {}
            pre_w1 = {}
            pre_w2 = {}
            for oc in range(2):
                t_ = gen.tile([128, 8, 128], BF16, tag="wo_c", bufs=2,
                              name=f"pre_wo{oc}")
                nc.gpsimd.dma_start(
                    out=t_[:],
                    in_=wotr[oc, :, :, :].rearrange("ic p j -> p ic j"))
                pre_wo[oc] = t_
            for fc in range(3):
                t_ = gen.tile([128, 8, 128], F32R, tag="w1_c", bufs=3,
                              name=f"pre_w1{fc}")
                nc.gpsimd.dma_start(
                    out=t_[:],
                    in_=w1tr[fc, :, :, :].rearrange("ic p j -> p ic j"))
                pre_w1[fc] = t_
            for half in range(2):
                t_ = gen.tile([128, 16, 128], BF16, tag="w2_c", bufs=2,
                              name=f"pre_w2{half}")
                nc.gpsimd.dma_start(
                    out=t_[:],
                    in_=w2tr[0, half * 16:(half + 1) * 16, :, :]
                    .rearrange("fc p j -> p fc j"))
                pre_w2[half] = t_

            # persistent V^T tiles: 16 l-chunks of [128l, 2*65]
            vt_tiles = []
            for lc in range(16):
                v_ = const.tile([128, 130], BF16, tag=f"vt{lc}")
                nc.gpsimd.memset(v_[:, 64:65], 1.0)
                nc.gpsimd.memset(v_[:, 129:130], 1.0)
                vt_tiles.append(v_)

            # =====================================================
            # Attention phase, per batch b
            # =====================================================
            for b in range(B):
                # ---- P1: Q/K/V projection for this core's 2 heads ----
                proj = {}
                for name in ("wq", "wk", "wv"):
                    proj[name] = qkvp.tile([128, S], BF16, tag=f"{name}_out",
                                           name=f"{name}_out_b{b}")
                for sc in range(4):
                    if b == 0 and sc == 0:
                        xt_t = warm_xt
                    else:
                        xt_t = []
                        for ic in range(8):
                            x_ = gen.tile([128, 512], BF16, tag="xtin", bufs=8)
                            nc.sync.dma_start(
                                out=x_[:],
                                in_=xt[ic * 128:(ic + 1) * 128, b,
                                       sc * 512:(sc + 1) * 512],
                            )
                            xt_t.append(x_)
                    for pi, name in enumerate(("wq", "wk", "wv")):
                        pmm = ps.tile([128, 512], F32, tag="pb2", bufs=3,
                                      padded_shape=[128, 1024])
                        for ic in range(8):
                            nc.tensor.matmul(
                                out=pmm[:],
                                lhsT=w_sb[name][:, ic, :],
                                rhs=xt_t[ic][:],
                                start=(ic == 0),
                                stop=(ic == 7),
                            )
                        # evict + bias (+implicit bf16 round)
                        nc.vector.tensor_scalar_add(
                            out=proj[name][:, sc * 512:(sc + 1) * 512],
                            in0=pmm[:],
                            scalar1=bqkv_sb[:, pi:pi + 1],
                        )

                # ---- P2: transpose V -> vt_tiles [l, (hl,v)] ----
                for lc in range(16):
                    ptr = ps.tile([128, 128], BF16, tag="pb1", bufs=2,
                                  padded_shape=[128, 512])
                    nc.tensor.transpose(
                        out=ptr[:],
                        in_=proj["wv"][:, lc * 128:(lc + 1) * 128],
                        identity=ident[:],
                    )
                    # scatter (hl,v) -> free offset hl*65+v
                    for hl in range(2):
                        nc.vector.tensor_copy(
                            out=vt_tiles[lc][:, hl * 65:hl * 65 + 64],
                            in_=ptr[:, hl * 64:(hl + 1) * 64],
                        )

                # ---- P3: attention per head ----
                for hl in range(2):
                    hb = hl * 64
                    for sc in range(4):
                        exp_t = []
                        for lc2 in range(8):
                            psc = ps.tile([128, 1024], F32, tag="pb2", bufs=3)
                            for k in range(2):
                                nc.tensor.matmul(
                                    out=psc[:, k * 512:(k + 1) * 512],
                                    lhsT=proj["wk"][hb:hb + 64,
                                                    (lc2 * 2 + k) * 128:
                                                    (lc2 * 2 + k + 1) * 128],
                                    rhs=proj["wq"][hb:hb + 64,
                                                   sc * 512:(sc + 1) * 512],
                                    start=True,
                                    stop=True,
                                )
                            e_ = gen.tile([128, 1024], BF16, tag="u2k",
                                          bufs=24)
                            nc.scalar.activation(
                                out=e_[:], in_=psc[:], func=AF.Exp
                            )
                            exp_t.append(e_)
                        pctx = ps.tile([65, 512], F32, tag="pb1", bufs=2,
                                       padded_shape=[128, 512])
                        for lc in range(16):
                            nc.tensor.matmul(
                                out=pctx[:],
                                lhsT=vt_tiles[lc][:, hl * 65:hl * 65 + 65],
                                rhs=exp_t[lc // 2][:, (lc % 2) * 512:
                                                   (lc % 2 + 1) * 512],
                                start=(lc == 0),
                                stop=(lc == 15),
                            )
                        dsum = gen.tile([1, 512], F32, tag="dsum", bufs=1)
                        nc.vector.tensor_copy(out=dsum[:], in_=pctx[64:65, :])
                        rf32 = gen.tile([1, 512], F32, tag="rf32", bufs=1)
                        nc.vector.reciprocal_approx_fast(
                            out=rf32[:], in_=dsum[:])
                        recip = gen.tile([1, 512], BF16, tag="recip", bufs=2)
                        nc.vector.tensor_copy(out=recip[:], in_=rf32[:])
                        pbc = ps.tile([64, 512], F32, tag="pb1", bufs=2,
                                      padded_shape=[128, 512])
                        nc.tensor.matmul(
                            out=pbc[:], lhsT=ones_1x64[:], rhs=recip[:],
                            start=True, stop=True,
                        )
                        bc_sb = gen.tile([64, 512], F32, tag="bc_sb", bufs=2)
                        nc.vector.tensor_copy(out=bc_sb[:], in_=pbc[:])
                        stage = gen.tile([64, 512], BF16, tag="stage", bufs=2)
                        nc.vector.tensor_tensor(
                            out=stage[:], in0=pctx[0:64, :], in1=bc_sb[:],
                            op=ALU.mult,
                        )
                        # write the two destination shards
                        for half in range(2):
                            d = sc * 2 + half
                            nc.sync.dma_start(
                                out=a2a_in[b][d, hl, :, :],
                                in_=stage[:, half * 256:(half + 1) * 256],
                            )

                # A2A for this batch (overlaps later batches' attention)
                nc.gpsimd.collective_compute(
                    "AllToAll",
                    ALU.bypass,
                    replica_groups=[list(range(NC))],
                    ins=[a2a_in[b][:].opt()],
                    outs=[a2a_out[b][:].opt()],
                )



            # =====================================================
            # P5: back half on the core's 1024 tokens (2 chunks of 512),
            # two chunks emitted interleaved so each LayerNorm's serial
            # scalar chain is covered by the other chunk's matmul work.
            # =====================================================
            def ln_stats(rt_tiles, tagn):
                psum_s = ps.tile([1, 512], F32, tag="pb1", bufs=2,
                                 padded_shape=[128, 512],
                                 name=f"psum_s_{tagn}")
                for ic in range(8):
                    nc.tensor.matmul(
                        out=psum_s[:], lhsT=ones_128x1[:], rhs=rt_tiles[ic][:],
                        start=(ic == 0), stop=(ic == 7),
                    )
                sq_t = []
                for ic in range(8):
                    sq = gen.tile([128, 512], F32R, tag="sq", bufs=2,
                                  name=f"sq_{tagn}_{ic}")
                    nc.scalar.activation(out=sq[:], in_=rt_tiles[ic][:],
                                         func=AF.Square)
                    sq_t.append(sq)
                psum_q = ps.tile([1, 512], F32, tag="pb1", bufs=2,
                                 padded_shape=[128, 512],
                                 name=f"psum_q_{tagn}")
                for ic in range(8):
                    nc.tensor.matmul(
                        out=psum_q[:], lhsT=ones_128x1[:], rhs=sq_t[ic][:],
                        start=(ic == 0), stop=(ic == 7),
                    )
                mu = gen.tile([1, 512], F32R, tag="mu", bufs=2,
                              name=f"mu_{tagn}")
                nc.vector.tensor_scalar_mul(out=mu[:], in0=psum_s[:],
                                            scalar1=1.0 / DM)
                ex2 = gen.tile([1, 512], F32, tag="stat", bufs=3,
                               name=f"ex2_{tagn}")
                nc.vector.tensor_scalar_mul(out=ex2[:], in0=psum_q[:],
                                            scalar1=1.0 / DM)
                musq = gen.tile([1, 512], F32, tag="stat", bufs=3,
                                name=f"musq_{tagn}")
                nc.vector.tensor_tensor(out=musq[:], in0=mu[:], in1=mu[:],
                                        op=ALU.mult)
                var = gen.tile([1, 512], F32, tag="stat", bufs=3,
                               name=f"var_{tagn}")
                nc.vector.tensor_tensor(out=var[:], in0=ex2[:], in1=musq[:],
                                        op=ALU.subtract)
                sd = gen.tile([1, 512], F32, tag="stat", bufs=3,
                              name=f"sd_{tagn}")
                nc.scalar.activation(out=sd[:], in_=var[:], func=AF.Sqrt,
                                     bias=eps_sb[:])
                rscr = gen.tile([1, 512], F32, tag="stat", bufs=3,
                                name=f"rscr_{tagn}")
                rf = gen.tile([1, 512], F32, tag="stat", bufs=3,
                              name=f"rf_{tagn}")
                nc.vector.reciprocal_approx_accurate(out=rf[:], in_=sd[:],
                                                     scratch=rscr[:])
                rstd = gen.tile([1, 512], F32R, tag="rstd", bufs=2,
                                name=f"rstd_{tagn}")
                nc.vector.tensor_copy(out=rstd[:], in_=rf[:])
                return mu, rstd

            def ln_norm(stats, rt_tiles, g_sb, b_sb, out_dtype, fold_gb,
                        tagn):
                mu, rstd = stats
                pmu = ps.tile([128, 512], F32, tag="pb1", bufs=2,
                              name=f"pmu_{tagn}")
                nc.tensor.matmul(out=pmu[:], lhsT=ones_1x128[:], rhs=mu[:],
                                 start=True, stop=True)
                prs = ps.tile([128, 512], F32, tag="pb1", bufs=2,
                              name=f"prs_{tagn}")
                nc.tensor.matmul(out=prs[:], lhsT=ones_1x128[:], rhs=rstd[:],
                                 start=True, stop=True)
                outs = []
                for ic in range(8):
                    tmp = gen.tile([128, 512], F32, tag="lnw", bufs=4,
                                   name=f"tmp_{tagn}_{ic}")
                    nc.vector.tensor_tensor(out=tmp[:], in0=rt_tiles[ic][:],
                                            in1=pmu[:], op=ALU.subtract)
                    if fold_gb:
                        o_ = gen.tile([128, 512], out_dtype, tag="u2k",
                                      bufs=24, name=f"z_{tagn}_{ic}")
                        nc.vector.tensor_tensor(out=o_[:], in0=tmp[:],
                                                in1=prs[:], op=ALU.mult)
                    else:
                        a_ = gen.tile([128, 512], F32, tag="lnw", bufs=4,
                                      name=f"a_{tagn}_{ic}")
                        nc.scalar.activation(out=a_[:], in_=prs[:],
                                             func=AF.Copy,
                                             scale=g_sb[:, ic:ic + 1])
                        t2 = gen.tile([128, 512], F32, tag="lnw", bufs=4,
                                      name=f"t2_{tagn}_{ic}")
                        nc.vector.tensor_tensor(out=t2[:], in0=tmp[:],
                                                in1=a_[:], op=ALU.mult)
                        o_ = gen.tile([128, 512], out_dtype, tag="u2k",
                                      bufs=24, name=f"o_{tagn}_{ic}")
                        nc.scalar.activation(out=o_[:], in_=t2[:],
                                             func=AF.Identity,
                                             bias=b_sb[:, ic:ic + 1])
                    outs.append(o_)
                return outs

            def emit_ctx_wo(t2c):
                b0 = t2c * 2
                ctx_t = []
                for ic in range(8):
                    c_ = gen.tile([128, 512], BF16, tag="ctx", bufs=8,
                                  name=f"c_{t2c}_{ic}")
                    for half in range(2):
                        nc.scalar.dma_start(
                            out=c_[:, half * 256:(half + 1) * 256],
                            in_=a2a_out[b0 + half][ic, :, :, :].rearrange(
                                "hl v s -> (hl v) s"
                            ),
                        )
                    ctx_t.append(c_)
                r1_t = []
                for oc in range(8):
                    if t2c == 0 and oc in pre_wo:
                        wo_c = pre_wo.pop(oc)
                    else:
                        wo_c = gen.tile([128, 8, 128], BF16, tag="wo_c",
                                        bufs=2, name=f"wo_c_{t2c}_{oc}")
                        nc.sync.dma_start(
                            out=wo_c[:],
                            in_=wotr[oc, :, :, :].rearrange(
                                "ic p j -> p ic j"),
                        )
                    pmm = ps.tile([128, 512], F32, tag="pb2", bufs=3,
                                  padded_shape=[128, 1024],
                                  name=f"pwo_{t2c}_{oc}")
                    for ic in range(8):
                        nc.tensor.matmul(
                            out=pmm[:], lhsT=wo_c[:, ic, :],
                            rhs=ctx_t[ic][:],
                            start=(ic == 0), stop=(ic == 7),
                        )
                    x_ = gen.tile([128, 512], F32, tag="xres", bufs=3,
                                  name=f"x_{t2c}_{oc}")
                    nc.sync.dma_start(
                        out=x_[:],
                        in_=xts[oc * 128:(oc + 1) * 128,
                                b0:b0 + 2, :].rearrange("p b s -> p (b s)"),
                    )
                    r1 = gen.tile([128, 512], F32R, tag="u2k", bufs=24,
                                  name=f"r1_{t2c}_{oc}")
                    nc.vector.scalar_tensor_tensor(
                        out=r1[:], in0=pmm[:], scalar=wob_sb[:, oc:oc + 1],
                        in1=x_[:], op0=ALU.add, op1=ALU.add,
                    )
                    r1_t.append(r1)
                return r1_t

            def emit_mlp1(t2c, x1_t):
                h_t = []
                for fc in range(32):
                    if t2c == 0 and fc in pre_w1:
                        w1_c = pre_w1.pop(fc)
                    else:
                        w1_c = gen.tile([128, 8, 128], F32R, tag="w1_c",
                                        bufs=3, name=f"w1_c_{t2c}_{fc}")
                        nc.sync.dma_start(
                            out=w1_c[:],
                            in_=w1tr[fc, :, :, :].rearrange(
                                "ic p j -> p ic j"),
                        )
                    pmm = ps.tile([128, 512], F32, tag="pb2", bufs=3,
                                  padded_shape=[128, 1024],
                                  name=f"ph_{t2c}_{fc}")
                    for ic in range(8):
                        nc.tensor.matmul(
                            out=pmm[:], lhsT=w1_c[:, ic, :], rhs=x1_t[ic][:],
                            start=(ic == 0), stop=(ic == 7),
                        )
                    h_ = gen.tile([128, 512], BF16, tag="ht", bufs=33,
                                  name=f"h_{t2c}_{fc}")
                    nc.scalar.activation(out=h_[:], in_=pmm[:], func=AF.Relu,
                                         bias=b1_sb[:, fc:fc + 1])
                    h_t.append(h_)
                return h_t

            def emit_mlp2(t2c, h_t, x1_t):
                r2_t = []
                for oc in range(8):
                    pmm = ps.tile([128, 512], F32, tag="pb2", bufs=3,
                                  padded_shape=[128, 1024],
                                  name=f"pm_{t2c}_{oc}")
                    for half in range(2):
                        if t2c == 0 and oc == 0 and half in pre_w2:
                            w2_c = pre_w2.pop(half)
                        else:
                            w2_c = gen.tile(
                                [128, 16, 128], BF16, tag="w2_c", bufs=2,
                                name=f"w2_c_{t2c}_{oc}_{half}")
                            nc.sync.dma_start(
                                out=w2_c[:],
                                in_=w2tr[oc, half * 16:(half + 1) * 16, :, :]
                                .rearrange("fc p j -> p fc j"),
                            )
                        for f2 in range(16):
                            fc = half * 16 + f2
                            nc.tensor.matmul(
                                out=pmm[:], lhsT=w2_c[:, f2, :],
                                rhs=h_t[fc][:],
                                start=(fc == 0), stop=(fc == 31),
                            )
                    r2p = gen.tile([128, 512], F32, tag="lnw", bufs=4,
                                   name=f"r2p_{t2c}_{oc}")
                    nc.vector.scalar_tensor_tensor(
                        out=r2p[:], in0=x1_t[oc][:],
                        scalar=ln_sb["ln1g"][:, oc:oc + 1],
                        in1=pmm[:], op0=ALU.mult, op1=ALU.add,
                    )
                    r2 = gen.tile([128, 512], F32R, tag="u2k", bufs=24,
                                  name=f"r2_{t2c}_{oc}")
                    nc.vector.tensor_scalar_add(
                        out=r2[:], in0=r2p[:], scalar1=b2_sb[:, oc:oc + 1],
                    )
                    r2_t.append(r2)
                return r2_t

            def emit_out(t2c, o_t):
                b0 = t2c * 2
                for oc in range(8):
                    nc.sync.dma_start(
                        out=ot[oc * 128:(oc + 1) * 128,
                               b0:b0 + 2, :].rearrange("p b s -> p (b s)"),
                        in_=o_t[oc][:],
                    )

            r1_0 = emit_ctx_wo(0)
            st1_0 = ln_stats(r1_0, "l1c0")
            r1_1 = emit_ctx_wo(1)
            st1_1 = ln_stats(r1_1, "l1c1")
            x1_0 = ln_norm(st1_0, r1_0, None, None, F32R, True, "l1c0")
            h_0 = emit_mlp1(0, x1_0)
            x1_1 = ln_norm(st1_1, r1_1, None, None, F32R, True, "l1c1")
            r2_0 = emit_mlp2(0, h_0, x1_0)
            st2_0 = ln_stats(r2_0, "l2c0")
            h_1 = emit_mlp1(1, x1_1)
            o_0 = ln_norm(st2_0, r2_0, ln_sb["ln2g"], ln_sb["ln2b"], F32,
                          False, "l2c0")
            emit_out(0, o_0)
            r2_1 = emit_mlp2(1, h_1, x1_1)
            st2_1 = ln_stats(r2_1, "l2c1")
            o_1 = ln_norm(st2_1, r2_1, ln_sb["ln2g"], ln_sb["ln2b"], F32,
                          False, "l2c1")
            emit_out(1, o_1)

    nc.compile()
    return nc


# ------------------------------------------------------------------
# Host side
# ------------------------------------------------------------------
def _get_runner():
    if "runner" in _CACHE:
        return _CACHE["runner"]
    import jax
    from jax.sharding import Mesh, PartitionSpec
    try:
        from jax.experimental.shard_map import shard_map
    except ImportError:
        from jax.shard_map import shard_map
    from concourse import bass2jax
    from concourse.bass2jax import _bass_exec_p, install_neuronx_cc_hook

    nc = _build_nc()
    install_neuronx_cc_hook()
    partition_name = nc.partition_id_tensor.name if nc.partition_id_tensor else None
    in_names, out_names, out_avals, zero_outs = [], [], [], []
    for alloc in nc.m.functions[0].allocations:
        if not isinstance(alloc, mybir.MemoryLocationSet):
            continue
        name = alloc.memorylocations[0].name
        if alloc.kind == "ExternalInput":
            if name != partition_name:
                in_names.append(name)
        elif alloc.kind == "ExternalOutput":
            out_names.append(name)
            shape = tuple(alloc.tensor_shape)
            dtype = mybir.dt.np(alloc.dtype)
            out_avals.append(jax.core.ShapedArray(shape, dtype))
            zero_outs.append(np.zeros(shape, dtype))
    n_params = len(in_names)
    all_in_names = list(in_names) + list(out_names)
    if partition_name is not None:
        all_in_names.append(partition_name)

    def _body(*args):
        operands = list(args)
        if partition_name is not None:
            operands.append(bass2jax.partition_id_tensor())
        outs = _bass_exec_p.bind(
            *operands,
            out_avals=tuple(out_avals),
            in_names=tuple(all_in_names),
            out_names=tuple(out_names),
            lowering_input_output_aliases=(),
            sim_require_finite=True,
            sim_require_nnan=True,
            nc=nc,
        )
        return tuple(outs)

    donate = tuple(range(n_params, n_params + len(out_names)))
    devices = jax.devices()[:NC]
    mesh = Mesh(np.asarray(devices), ("core",))
    in_specs = (PartitionSpec("core"),) * (n_params + len(out_names))
    out_specs = (PartitionSpec("core"),) * len(out_names)
    fn = jax.jit(
        shard_map(_body, mesh=mesh, in_specs=in_specs, out_specs=out_specs,
                  check_rep=False),
        donate_argnums=donate, keep_unused=True,
    )

    class R:
        pass

    r = R()
    r.fn = fn
    r.in_names = in_names
    r.out_names = out_names
    r.out_avals = out_avals
    _CACHE["runner"] = r
    return r


def _prep_in_maps(X, WQ_w, WQ_b, WK_w, WK_b, WV_w, WV_b, WO_w, WO_b,
                  ln1_g, ln1_b, W1, b1, W2, b2, ln2_g, ln2_b):
    import ml_dtypes
    f = np.float32
    bf = ml_dtypes.bfloat16
    XT = np.ascontiguousarray(X.transpose(2, 1, 0)).astype(f)  # [DM,B,S]
    wotr = np.ascontiguousarray(
        WO_w.reshape(8, 128, 8, 128).transpose(0, 2, 3, 1)).astype(bf)
    W1f = (W1 * ln1_g[None, :]).astype(np.float64)
    b1f = (b1 + W1 @ ln1_b).astype(f)
    w1tr = np.ascontiguousarray(
        W1f.reshape(32, 128, 8, 128).transpose(0, 2, 3, 1)).astype(f)
    w2tr = np.ascontiguousarray(
        W2.reshape(8, 128, 32, 128).transpose(0, 2, 3, 1)).astype(bf)
    wob_t = np.ascontiguousarray(WO_b.reshape(8, 128).T).astype(f)
    b1_t = np.ascontiguousarray(b1f.reshape(32, 128).T).astype(f)
    b2f = (b2 + ln1_b).astype(f)
    b2_t = np.ascontiguousarray(b2f.reshape(8, 128).T).astype(f)
    ln1g_t = np.ascontiguousarray(ln1_g.reshape(8, 128).T).astype(f)
    ln1b_t = np.ascontiguousarray(ln1_b.reshape(8, 128).T).astype(f)
    ln2g_t = np.ascontiguousarray(ln2_g.reshape(8, 128).T).astype(f)
    ln2b_t = np.ascontiguousarray(ln2_b.reshape(8, 128).T).astype(f)

    in_maps = []
    for c in range(NC):
        h0 = 2 * c
        # [2,DK,DM] -> [DM, 128]: W2h[j, hl*64+k] = W[h0+hl, k, j]
        wq2 = WQ_w[h0:h0 + 2].reshape(128, DM).T / 8.0
        wk2 = WK_w[h0:h0 + 2].reshape(128, DM).T
        wv2 = WV_w[h0:h0 + 2].reshape(128, DM).T
        # [8,128,128] layout: [ic, p, j] = W2h[ic*128+p, j]
        wqt = np.ascontiguousarray(wq2.reshape(8, 128, 128)).astype(bf)
        wkt = np.ascontiguousarray(wk2.reshape(8, 128, 128)).astype(bf)
        wvt = np.ascontiguousarray(wv2.reshape(8, 128, 128)).astype(bf)
        bq = WQ_b[h0:h0 + 2].reshape(128) / 8.0
        bk = WK_b[h0:h0 + 2].reshape(128)
        bv = WV_b[h0:h0 + 2].reshape(128)
        bqkv = np.stack([bq, bk, bv], axis=1).astype(f)
        in_maps.append({
            "xt": XT.astype(bf),
            "xts": np.ascontiguousarray(XT[:, :, c * SS:(c + 1) * SS]),
            "wqt": wqt, "wkt": wkt, "wvt": wvt, "bqkv": bqkv,
            "wotr": wotr, "wob": wob_t,
            "w1tr": w1tr, "b1": b1_t, "w2tr": w2tr, "b2": b2_t,
            "ln1g": ln1g_t, "ln1b": ln1b_t, "ln2g": ln2g_t, "ln2b": ln2b_t,
        })
    return in_maps


def run_in_maps(in_maps):
    """Run the compiled kernel on prepared in_maps; returns list of out dicts."""
    import jax
    r = _get_runner()
    n = NC
    per_core = [[np.asarray(m[name]) for name in r.in_names] for m in in_maps]
    concat_in = [
        np.concatenate([per_core[c][i] for c in range(n)], axis=0)
        for i in range(len(r.in_names))
    ]
    concat_zeros = [
        np.zeros((n * a.shape[0], *a.shape[1:]), a.dtype) for a in r.out_avals
    ]
    out_arrs = r.fn(*concat_in, *concat_zeros)
    out_arrs = [np.asarray(a) for a in out_arrs]
    return [
        {name: out_arrs[i].reshape(n, *r.out_avals[i].shape)[c]
         for i, name in enumerate(r.out_names)}
        for c in range(n)
    ]


def kernel(**inputs):
    in_maps = _prep_in_maps(**inputs)
    results = run_in_maps(in_maps)
    # assemble: each core's ot is [DM, B, SS] covering s in [c*SS,(c+1)*SS)
    ot_full = np.concatenate([results[c]["ot"] for c in range(NC)], axis=2)
    # [DM, B, S] -> [S, B, DM]
    return np.ascontiguousarray(ot_full.transpose(2, 1, 0))
